# revision 41
# baseline (speedup 1.0000x reference)
"""Trainium2 Bass kernel for a 16-head attention block (dense_transformer).

Computation (per reference):
    q = y2_for @ Wq + bq ; k = y2_back @ Wk + bk ; v = (y2_for+y2_back) @ Wv + bv
    attn = softmax(q k^T / sqrt(d)) ; out = (attn @ v + y2_for + y2_back) @ Wo + bo

Sharding: 8 cores = 2 batches x 4 head-groups (4 heads / 256 dims each).
Each core computes its batch's attention for its 4 heads plus the partial
output projection  (attn_slice + resid_slice) @ Wo[slice, :];  the host sums
the 4 partials per batch and adds bo.  No cross-device communication.

Device structure (single program, SPMD over 8 cores):
  - S_T orientation ([key j on partitions, query i free]) so the P@V matmul
    contracts j on partitions and softmax row-sums come free from a 65th
    all-ones column in the V weight tile.
  - exp() on ScalarE (the bottleneck engine: 16.8M softmax elements/core)
    with the 1/sqrt(d) scale folded into the activation's free affine
    pre-scale; scores are ~N(0,1) so fp32 exp needs no max-subtraction.
  - One shared PSUM pool for the whole kernel (tags: st=2x2 banks score
    tiles, aux=2x1 banks projection chains, po=2x1 banks PV accumulators =
    8 banks) so projection/V/output-projection PE work pipelines under the
    ACT-bound exp stream instead of serializing phase by phase.
  - (y2_for+y2_back) is computed on device (DVE) in small staged tiles; the
    residual slice arrives as a separate small input.
"""

import numpy as np

B, N, DIM, HEADS = 2, 2048, 1024, 16
DH = DIM // HEADS  # 64
P = 128
NG = 4  # head-group shards (cores per batch)
CW = DIM // NG  # 256 columns per core
KO = DIM // P  # 8 contraction tiles
NB = N // P  # 16 n/j tiles
IB = N // 512  # 4 i-blocks of 512

_cache = {}


def _build(repeat=1, debug_dump=False):
    from contextlib import ExitStack

    import concourse.mybir as mybir
    import concourse.tile as tile
    from concourse import bacc

    f32 = mybir.dt.float32
    bf16 = mybir.dt.bfloat16

    nc = bacc.Bacc("TRN2", target_bir_lowering=False, debug=False,
                   enable_asserts=False)

    at = nc.dram_tensor("at", [DIM, N], bf16, kind="ExternalInput")
    bt = nc.dram_tensor("bt", [DIM, N], bf16, kind="ExternalInput")
    ctr = nc.dram_tensor("ctr", [CW, N], bf16, kind="ExternalInput")
    wq = nc.dram_tensor("wq", [DIM, CW], bf16, kind="ExternalInput")
    wk = nc.dram_tensor("wk", [DIM, CW], bf16, kind="ExternalInput")
    wv = nc.dram_tensor("wv", [DIM, CW], bf16, kind="ExternalInput")
    wo = nc.dram_tensor("wo", [CW, DIM], bf16, kind="ExternalInput")
    bq = nc.dram_tensor("bq", [CW], f32, kind="ExternalInput")
    bk = nc.dram_tensor("bk", [CW], f32, kind="ExternalInput")
    bv4 = nc.dram_tensor("bv4", [P, 2 * CW], f32, kind="ExternalInput")
    outT = nc.dram_tensor("outT", [DIM, N], f32, kind="ExternalOutput")
    dbg = None
    if debug_dump:
        dbg = {
            "d_qt": nc.dram_tensor("d_qt", [P, 2, N], bf16, kind="ExternalOutput"),
            "d_kt": nc.dram_tensor("d_kt", [P, 2, N], bf16, kind="ExternalOutput"),
            "d_v": nc.dram_tensor("d_v", [P, NB, 4, DH + 1], bf16,
                                  kind="ExternalOutput"),
            "d_x": nc.dram_tensor("d_x", [P, 2, N], bf16, kind="ExternalOutput"),
            "d_pp": nc.dram_tensor("d_pp", [P, 4, 2, 512], bf16,
                                   kind="ExternalOutput"),
            "d_po": nc.dram_tensor("d_po", [2, DH + 1, 512], f32,
                                   kind="ExternalOutput"),
            "d_ri": nc.dram_tensor("d_ri", [2, 1, 512], f32,
                                   kind="ExternalOutput"),
            "d_rb": nc.dram_tensor("d_rb", [2, DH, 512], f32,
                                   kind="ExternalOutput"),
        }

    scale = float(DH) ** -0.5

    with tile.TileContext(nc) as tc, ExitStack() as ctx:
        for _rep in range(repeat):
            _emit_body(nc, tc, ctx, mybir,
                       at, bt, ctr, wq, wk, wv, wo, bq, bk, bv4, outT,
                       scale, _rep, dbg)
    nc.compile()

    return nc


def _emit_body(nc, tc, _outer_ctx, mybir, at, bt, ctr, wq, wk, wv, wo, bq, bk,
               bv4, outT, scale, rep, dbg=None):
    from contextlib import ExitStack

    f32 = mybir.dt.float32
    bf16 = mybir.dt.bfloat16
    AF = mybir.ActivationFunctionType
    ADD = mybir.AluOpType.add
    MUL = mybir.AluOpType.mult
    with ExitStack() as ctx:
        const = ctx.enter_context(tc.tile_pool(name=f"const{rep}", bufs=1))
        acts = ctx.enter_context(tc.tile_pool(name=f"acts{rep}", bufs=1))
        ctq_pool = ctx.enter_context(tc.tile_pool(name=f"ctq{rep}", bufs=3))
        stage = ctx.enter_context(tc.tile_pool(name=f"stage{rep}", bufs=4))
        rin = ctx.enter_context(tc.tile_pool(name=f"rin{rep}", bufs=2))
        ppool = ctx.enter_context(tc.tile_pool(name=f"ppool{rep}", bufs=1))
        psum = ctx.enter_context(tc.tile_pool(name=f"psum{rep}", bufs=2,
                                              space="PSUM"))

        # --- persistent SBUF tensors -----------------------------------
        wq_sb = const.tile([P, KO, CW], bf16, tag="wq")
        wk_sb = const.tile([P, KO, CW], bf16, tag="wk")
        wv_sb = const.tile([P, KO, CW], bf16, tag="wv")
        wo_sb = const.tile([P, 2, DIM], bf16, tag="wo")
        bq_sb = const.tile([P, 2], f32, tag="bq")
        bk_sb = const.tile([P, 2], f32, tag="bk")
        bv_bc = const.tile([P, 2 * CW], f32, tag="bvb")
        qt_sb = const.tile([P, 2, N], bf16, tag="qt")  # Q.T  [256, 2048]
        kt_sb = const.tile([P, 2, N], bf16, tag="kt")  # K.T
        x_sb = const.tile([P, 2, N], bf16, tag="x")    # (O/r + resid).T
        ctr_sb = const.tile([P, 2, N], bf16, tag="ctr")  # resid slice .T
        v_sb = const.tile([P, NB, 4, DH + 1], bf16, tag="v")  # V + ones col
        at_sb = acts.tile([P, KO, N], bf16, tag="at")
        bt_sb = acts.tile([P, KO, N], bf16, tag="bt")

        # preload the exp activation table (one-time ~2.7us)
        warm = const.tile([1, 8], f32, tag="warm")
        nc.vector.memset(warm[:], 0.0)
        warm2 = const.tile([1, 8], f32, tag="warm2")
        nc.scalar.activation(warm2[:], warm[:], AF.Exp)

        # --- input DMAs: small tensors first, then at/bt i-block-major --
        nc.sync.dma_start(bq_sb[:], bq.ap().rearrange("(m p) -> p m", p=P))
        nc.sync.dma_start(bk_sb[:], bk.ap().rearrange("(m p) -> p m", p=P))
        nc.sync.dma_start(bv_bc[:], bv4.ap())
        nc.sync.dma_start(wq_sb[:], wq.ap().rearrange("(ko p) m -> p ko m", p=P))
        nc.sync.dma_start(wk_sb[:], wk.ap().rearrange("(ko p) m -> p ko m", p=P))
        nc.sync.dma_start(wv_sb[:], wv.ap().rearrange("(ko p) m -> p ko m", p=P))
        at_r = at.ap().rearrange("(ko p) n -> p ko n", p=P)
        bt_r = bt.ap().rearrange("(ko p) n -> p ko n", p=P)
        for ib in range(IB):
            sl = slice(ib * 512, (ib + 1) * 512)
            nc.sync.dma_start(at_sb[:, :, sl], at_r[:, :, sl])
            nc.sync.dma_start(bt_sb[:, :, sl], bt_r[:, :, sl])
        nc.sync.dma_start(ctr_sb[:], ctr.ap().rearrange("(m p) n -> p m n", p=P))
        nc.sync.dma_start(wo_sb[:], wo.ap().rearrange("(kt p) d -> p kt d", p=P))
        nc.vector.memset(v_sb[:, :, :, DH:DH + 1], 1.0)

        # --- emission helpers ------------------------------------------
        def proj_T(dst, w_sb, src_sb, bias_sb, mb, ib):
            # dst[:, mb, ib*512:+512] = (W[:, mb-slice].T @ src).T + bias
            pq = psum.tile([P, 512], f32, tag="aux", name=f"pj{mb}{ib}")
            for ko in range(KO):
                nc.tensor.matmul(
                    pq[:],
                    lhsT=w_sb[:, ko, mb * P:(mb + 1) * P],
                    rhs=src_sb[:, ko, ib * 512:(ib + 1) * 512],
                    start=(ko == 0), stop=(ko == KO - 1))
            nc.vector.tensor_scalar_add(
                dst[:, mb, ib * 512:(ib + 1) * 512], pq[:], bias_sb[:, mb:mb + 1])

        def v_prod(q):
            # V rows for n-blocks (2q, 2q+1); ct staged on device
            csl = slice(q * 2 * P, (q + 1) * 2 * P)
            ctq = ctq_pool.tile([P, KO, 2 * P], bf16, tag="ctq", name="ctq")
            nc.vector.tensor_tensor(ctq[:], at_sb[:, :, csl], bt_sb[:, :, csl],
                                    ADD)
            pv = psum.tile([P, 512], f32, tag="aux", name=f"pv{q}")
            for half in range(2):
                for ko in range(KO):
                    nc.tensor.matmul(
                        pv[:, half * CW:(half + 1) * CW],
                        lhsT=ctq[:, ko, half * P:(half + 1) * P],
                        rhs=wv_sb[:, ko, :],
                        start=(ko == 0), stop=(ko == KO - 1))
            nc.vector.tensor_tensor(
                v_sb[:, 2 * q:2 * q + 2, :, 0:DH],
                pv[:].rearrange("p (b h d) -> p b h d", h=4, d=DH),
                bv_bc[:].rearrange("p (b h d) -> p b h d", h=4, d=DH),
                ADD)

        PPC = 4  # jt tiles per p-chunk

        def qk_exp_chunk(pair, ib, cnk, pch):
            # scores+exp for jt in [4*cnk, 4*cnk+4)
            isl = slice(ib * 512, ib * 512 + 512)
            for j in range(PPC):
                jt = PPC * cnk + j
                st = psum.tile([P, 2, 512], f32, tag="st", name="st")
                for h in range(2):
                    nc.tensor.matmul(
                        st[:, h, :],
                        lhsT=kt_sb[h * DH:(h + 1) * DH, pair, jt * P:(jt + 1) * P],
                        rhs=qt_sb[h * DH:(h + 1) * DH, pair, isl],
                        start=True, stop=True)
                nc.scalar.activation(pch[:, j, :, :], st[:], AF.Exp,
                                     scale=scale)

        def pv_chunk(pair, cnk, pch, po):
            for j in range(PPC):
                jt = PPC * cnk + j
                for h in range(2):
                    nc.tensor.matmul(
                        po[h][:],
                        lhsT=v_sb[:, jt, pair * 2 + h, :],
                        rhs=pch[:, j, h, :],
                        start=(jt == 0), stop=(jt == NB - 1))

        def norm_resid(pair, ib, po):
            isl = slice(ib * 512, ib * 512 + 512)
            for h in range(2):
                # custom-DVE recip can't read PSUM on HW; copy rowsum to SBUF
                rsum = rin.tile([1, 512], f32, tag="rs", name="rs")
                nc.vector.tensor_copy(out=rsum[:], in_=po[h][DH:DH + 1, :])
                rinv = rin.tile([1, 512], f32, tag="ri", name="ri")
                nc.vector.reciprocal_approx_fast(rinv[:], rsum[:])
                rb = rin.tile([DH, 512], f32, tag="rb", name="rb")
                nc.gpsimd.partition_broadcast(rb[:], rinv[:], channels=DH)
                if dbg is not None and (pair, ib) == (1, 3):
                    pod = rin.tile([DH + 1, 512], f32, tag="pod", name="pod")
                    nc.vector.tensor_copy(out=pod[:], in_=po[h][:])
                    nc.sync.dma_start(dbg["d_po"].ap()[h], pod[:])
                    nc.sync.dma_start(dbg["d_ri"].ap()[h], rinv[:])
                    nc.sync.dma_start(dbg["d_rb"].ap()[h], rb[:])
                nc.vector.tensor_tensor(
                    x_sb[h * DH:(h + 1) * DH, pair, isl],
                    po[h][0:DH, :], rb[:], MUL)
            nc.vector.tensor_tensor(
                x_sb[:, pair, isl], x_sb[:, pair, isl],
                ctr_sb[:, pair, isl], ADD)

        def combo(pair, ib):
            po = [psum.tile([DH + 1, 512], f32, tag="po", name=f"po{h}")
                  for h in range(2)]
            for cnk in range(NB // PPC):
                pch = ppool.tile([P, PPC, 2, 512], bf16, tag="pp", name="pp")
                qk_exp_chunk(pair, ib, cnk, pch)
                if dbg is not None and (pair, ib, cnk) == (1, 3, 3):
                    nc.sync.dma_start(dbg["d_pp"].ap(), pch[:])
                pv_chunk(pair, cnk, pch, po)
            norm_resid(pair, ib, po)

        def outproj(dc, ib):
            pout = psum.tile([P, 512], f32, tag="aux", name=f"po3{dc}{ib}")
            for kt in range(2):
                nc.tensor.matmul(
                    pout[:],
                    lhsT=wo_sb[:, kt, dc * P:(dc + 1) * P],
                    rhs=x_sb[:, kt, ib * 512:(ib + 1) * 512],
                    start=(kt == 0), stop=(kt == 1))
            ot = stage.tile([P, 512], f32, tag="ot", name="ot")
            nc.any.tensor_copy(out=ot[:], in_=pout[:])
            nc.sync.dma_start(
                outT.ap()[dc * P:(dc + 1) * P, ib * 512:(ib + 1) * 512], ot[:])

        # --- emission order: pipeline attention with everything else ---
        # First combo interleaves the mb0 projections and V production per
        # i-block so PE's scheduled stream never stalls on not-yet-arrived
        # at/bt DMA slices, and exp starts as soon as the first Q.T/K.T
        # chains land.  Pair-0 combos run first so mb1 projections and V
        # have several combo windows of slack; pair-1 combos and the
        # output projections trail.
        po00 = [psum.tile([DH + 1, 512], f32, tag="po", name=f"po{h}")
                for h in range(2)]
        for ibg in range(IB):
            proj_T(qt_sb, wq_sb, at_sb, bq_sb, 0, ibg)
            proj_T(kt_sb, wk_sb, bt_sb, bk_sb, 0, ibg)
            pch = ppool.tile([P, PPC, 2, 512], bf16, tag="pp", name="pp")
            qk_exp_chunk(0, 0, ibg, pch)
            v_prod(2 * ibg)
            v_prod(2 * ibg + 1)
            pv_chunk(0, ibg, pch, po00)
        norm_resid(0, 0, po00)
        combo(0, 1)
        for ib in range(IB):
            proj_T(qt_sb, wq_sb, at_sb, bq_sb, 1, ib)
            proj_T(kt_sb, wk_sb, bt_sb, bk_sb, 1, ib)
        combo(0, 2)
        combo(0, 3)
        for ib in range(IB):
            combo(1, ib)
            for dc in range(DIM // P):
                outproj(dc, ib)
        if dbg is not None:
            nc.sync.dma_start(dbg["d_qt"].ap(), qt_sb[:])
            nc.sync.dma_start(dbg["d_kt"].ap(), kt_sb[:])
            nc.sync.dma_start(dbg["d_v"].ap(), v_sb[:])
            nc.sync.dma_start(dbg["d_x"].ap(), x_sb[:])

    return nc


def _get_nc(repeat=1):
    key = f"nc{repeat}"
    if key not in _cache:
        _cache[key] = _build(repeat)
    return _cache[key]


def _prep_in_maps(y2_for, y2_back, Wq, bq, Wk, bk, Wv, bv, Wo):
    import ml_dtypes
    bf16 = ml_dtypes.bfloat16

    y2_for = np.asarray(y2_for, dtype=np.float32)
    y2_back = np.asarray(y2_back, dtype=np.float32)
    in_maps = []
    for core in range(8):
        b, g = divmod(core, NG)
        c0 = g * CW
        ctr = (y2_for[b, :, c0:c0 + CW] + y2_back[b, :, c0:c0 + CW]).T
        bv_s = np.asarray(bv, dtype=np.float32)[c0:c0 + CW]
        in_maps.append({
            "at": np.ascontiguousarray(y2_for[b].T).astype(bf16),
            "bt": np.ascontiguousarray(y2_back[b].T).astype(bf16),
            "ctr": np.ascontiguousarray(ctr).astype(bf16),
            "wq": np.ascontiguousarray(np.asarray(Wq)[:, c0:c0 + CW]).astype(bf16),
            "wk": np.ascontiguousarray(np.asarray(Wk)[:, c0:c0 + CW]).astype(bf16),
            "wv": np.ascontiguousarray(np.asarray(Wv)[:, c0:c0 + CW]).astype(bf16),
            "wo": np.ascontiguousarray(np.asarray(Wo)[c0:c0 + CW, :]).astype(bf16),
            "bq": np.ascontiguousarray(np.asarray(bq, dtype=np.float32)[c0:c0 + CW]),
            "bk": np.ascontiguousarray(np.asarray(bk, dtype=np.float32)[c0:c0 + CW]),
            "bv4": np.ascontiguousarray(
                np.broadcast_to(np.tile(bv_s, 2), (P, 2 * CW))),
        })
    return in_maps


def _combine(results, bo):
    out = np.zeros((B, N, DIM), dtype=np.float32)
    for core in range(8):
        b = core // NG
        out[b] += results[core]["outT"].T
    out += np.asarray(bo, dtype=np.float32)
    return out


def run(y2_for, y2_back, Wq, bq, Wk, bk, Wv, bv, Wo, bo, repeat=1,
        **spmd_kwargs):
    """Full pipeline; returns (output, BassKernelResults)."""
    from concourse.bass_utils import run_bass_kernel_spmd

    nc = _get_nc(repeat)
    in_maps = _prep_in_maps(y2_for, y2_back, Wq, bq, Wk, bk, Wv, bv, Wo)
    res = run_bass_kernel_spmd(nc, in_maps, core_ids=list(range(8)),
                               **spmd_kwargs)
    return _combine(res.results, bo), res


def kernel(y2_for, y2_back, Wq, bq, Wk, bk, Wv, bv, Wo, bo):
    out, _ = run(y2_for, y2_back, Wq, bq, Wk, bk, Wv, bv, Wo, bo)
    return out
